# revision 23
# baseline (speedup 1.0000x reference)
"""Trainium2 Bass kernel for nn_Equilibrium (scatter_memory).

Computation (reference):
    x_out[t, 4m+0..3] = [x[t,0,m], x[t,2,m], x[t,2,m], x[t,1,m]]   # [T, 4M]
    f = einsum('ti,nio->nto', x_out, weight)                       # [N, T, 2]
    f_in = f[node_in - 1]                                          # [N_in, T, 2]
    f_b  = boundary sums over top/bottom/left/right                # [6, T, 1]

Strategy: only nodes referenced by node_in/top/bottom/left/right need f.
Host dedupes those indices (~5.7k unique of 10k), shards the unique set
across 8 cores, and pre-lays-out each core's weight rows as a transposed
[k', n] matrix so the contraction dim sits on SBUF partitions with fully
contiguous DMA.  The device streams its weight shard (memory-bound) through
TensorE:  f32 values are split into bf16 hi+lo parts on host; the PE
computes all four cross products at bf16 rate into fp32 PSUM, giving
~fp32 accuracy.  Host reassembles f_in (gather) and the 6 tiny boundary
sums from the per-core outputs.
"""

import os
import sys
import types

import numpy as np
import ml_dtypes

import concourse.bass as bass
import concourse.mybir as mybir
import concourse.tile as tile
from concourse.vector_clock import ScopedClock
from concourse.bass_utils import run_bass_kernel_spmd

N_CORES = 8
BF16 = ml_dtypes.bfloat16

# mode: "hilo" (bf16 hi/lo split, ~fp32 accurate), "f32", "f32r"
MODE = os.environ.get("EQ_KERNEL_MODE", "hilo")
CHUNKS_PER_DMA = int(os.environ.get("EQ_CHUNKS_PER_DMA", "8"))
TRACE = bool(int(os.environ.get("EQ_KERNEL_TRACE", "0")))
WP_BUFS = int(os.environ.get("EQ_WP_BUFS", "4"))
LAST_EXEC_NS = None
LAST_RESULTS = None

# ---------------------------------------------------------------------------
# Patch 1: the Tile tail drain may carry more sem waits than walrus's
# per-instruction cap (1 for CTRL/Drain in this toolchain). Split them.
_DRAIN_MAX_WAITS = 1


def _patched_drain_and_barrier(self, tick_clock, wait_clock):
    nc = self.nc
    drain_inst = nc.sync.drain()
    wait_clock.add_sem_waits(
        drain_inst.ins, ScopedClock({None: tick_clock.global_clock})
    )
    si = drain_inst.ins.sync_info
    if si is not None and len(si.on_wait) > _DRAIN_MAX_WAITS:
        waits = list(si.on_wait)
        si.on_wait = waits[:_DRAIN_MAX_WAITS]
        for i in range(_DRAIN_MAX_WAITS, len(waits), _DRAIN_MAX_WAITS):
            extra = nc.sync.drain()
            extra.ins.sync_info = mybir.SyncInfo(
                on_wait=list(waits[i : i + _DRAIN_MAX_WAITS]), on_update=[]
            )
    nc.all_engine_barrier()
    assert self.sems is not None
    popped = nc._tile_sem_poison_stack.pop()
    assert popped is self._sem_poison
    nc.clear_and_free_semaphores(list(self.sems.allocated().values()))
    nc.all_engine_barrier()


tile.TileContext._drain_and_barrier = _patched_drain_and_barrier


def _split_waits(nc, cap=_DRAIN_MAX_WAITS):
    """Walrus in this toolchain accepts at most one sem wait per
    instruction; hoist extra waits onto preceding same-engine NoOps."""
    n_id = 0
    for fn in nc.m.functions:
        for bb in fn.blocks:
            insts = list(bb.instructions)
            out = []
            changed = False
            for inst in insts:
                si = inst.sync_info
                if si is not None and len(si.on_wait) > cap:
                    waits = list(si.on_wait)
                    n_extra = len(waits) - cap
                    for i in range(0, n_extra, cap):
                        nop = mybir.InstNoOp(
                            name=f"waitsplit_{n_id}",
                            engine=inst.engine,
                            bass_nofuse=True,
                            sync_info=mybir.SyncInfo(
                                on_wait=list(waits[i : i + cap]), on_update=[]
                            ),
                        )
                        n_id += 1
                        nc.register_instruction(nop)
                        out.append(nop)
                    si.on_wait = waits[n_extra:]
                    changed = True
                out.append(inst)
            if changed:
                bb.instructions = out


# Patch 2: NTFF profile hook (only needed when tracing; the image's antenv
# lacks axon_hooks, so register a ctypes-based hook ourselves).
def _install_ntff_hook():
    try:
        from antenv.axon_hooks import get_axon_ntff_profile_hook  # noqa: F401

        return
    except ImportError:
        pass
    try:
        from trn_agent_boot.trn_boot import _ntff_profile_via_ctypes

        hook = _ntff_profile_via_ctypes("/opt/axon/libaxon_pjrt.so")
    except Exception:
        hook = None
    import antenv

    mod = types.ModuleType("antenv.axon_hooks")
    mod.get_axon_ntff_profile_hook = lambda: hook
    sys.modules["antenv.axon_hooks"] = mod
    antenv.axon_hooks = mod


# ---------------------------------------------------------------------------
# Device kernel builder


def _build_nc(Uc, mode, chunks_per_dma, wp_bufs=WP_BUFS):
    """Build the per-core Bass program.

    hilo mode stacks W_hi then W_lo along the contraction axis (128
    k-chunks; the stationary x2 = [x_hi | x_lo] repeats for both halves),
    so PSUM accumulates all four bf16 cross products: psum[0:32] =
    x_hi@(W_hi+W_lo), psum[32:64] = x_lo@(W_hi+W_lo); one DVE add
    finishes the job.

    Inputs (per core):
      w  : [G, 128, A*Uc]  moving operand (weight shard, transposed)
           G*A = n_chunks k-chunks of 128 rows; Uc columns each.
      x2 : [128, 64*M]     stationary operand; chunk kb uses cols (kb%64)*M.
    Output:
      f  : [32, Uc] f32    f[o*16+t, u]
    """
    A = chunks_per_dma
    if mode == "hilo":
        M = 64
        n_chunks = 128
        dt_in = mybir.dt.bfloat16
    elif mode == "f32":
        M = 32
        n_chunks = 64
        dt_in = mybir.dt.float32
    elif mode == "f32r":
        M = 32
        n_chunks = 64
        dt_in = mybir.dt.float32r
    elif mode == "fp16":
        M = 32
        n_chunks = 64
        dt_in = mybir.dt.float16
    elif mode == "mix3":
        return _build_nc_mix3(Uc, chunks_per_dma, wp_bufs)
    else:
        raise ValueError(mode)
    G = n_chunks // A
    C = Uc
    NB = 512
    n_blocks = (C + NB - 1) // NB

    nc = bass.Bass("TRN2", target_bir_lowering=False, debug=False,
                   num_devices=N_CORES)
    w = nc.dram_tensor("w", [G, 128, A * C], dt_in, kind="ExternalInput").ap()
    x2 = nc.dram_tensor("x2", [128, 64 * M], dt_in,
                        kind="ExternalInput").ap()
    f = nc.dram_tensor("f", [32, Uc], mybir.dt.float32,
                       kind="ExternalOutput").ap()

    with tile.TileContext(nc) as tc:
        with (
            tc.tile_pool(name="x2p", bufs=1) as x2p,
            tc.tile_pool(name="wp", bufs=wp_bufs) as wp,
            tc.tile_pool(name="psum", bufs=1, space="PSUM") as pp,
            tc.tile_pool(name="outp", bufs=1) as outp,
        ):
            x2_t = x2p.tile([128, 64 * M], dt_in)
            nc.sync.dma_start(x2_t[:], x2[:])

            psums = [pp.tile([M, min(NB, C - nb * NB)], mybir.dt.float32,
                             name=f"ps{nb}")
                     for nb in range(n_blocks)]
            out_t = outp.tile([32, Uc], mybir.dt.float32)

            for g in range(G):
                w_t = wp.tile([128, A * C], dt_in, tag="w")
                # alternate the two HWDGE queues (ACT / SP); w0 goes on ACT
                # so it transfers concurrently with x2 (which is on SP).
                eng = nc.scalar if g % 2 == 0 else nc.sync
                eng.dma_start(w_t[:], w[g, :, :])
                for a in range(A):
                    kb = g * A + a
                    j = kb % 64
                    lhsT = x2_t[:, j * M : (j + 1) * M]
                    for nb in range(n_blocks):
                        w_nb = min(NB, C - nb * NB)
                        rhs = w_t[:, a * C + nb * NB : a * C + nb * NB + w_nb]
                        nc.tensor.matmul(
                            psums[nb][:, :],
                            lhsT,
                            rhs,
                            start=(kb == 0),
                            stop=(kb == n_chunks - 1),
                        )

            # Combine psum blocks into f32 output [32, Uc].
            for nb in range(n_blocks):
                w_nb = min(NB, C - nb * NB)
                sl = slice(nb * NB, nb * NB + w_nb)
                if mode == "hilo":
                    nc.vector.tensor_copy(out_t[:, sl], psums[nb][0:32, :])
                    nc.vector.tensor_add(out_t[:, sl], out_t[:, sl],
                                         psums[nb][32:64, :])
                else:
                    nc.vector.tensor_copy(out_t[:, sl], psums[nb][:, :])
            nc.sync.dma_start(f[:], out_t[:])
    _split_waits(nc)
    return nc


MIX3_SCALE = 2.0 ** 18


def _build_nc_mix3(Uc, chunks_per_dma, wp_bufs):
    """3-byte mixed precision: W = fp16(W) + 2^-18 * fp8e4m3(scaled resid).

    hi half: 64 fp16 chunks, stationary [x_hi | x_lo] fp16 (M=64).
    lo half: 64 fp8e4m3 chunks, stationary x fp8e4m3 (M=32), psum scaled
    by 2^-18 during the combine.
    """
    A = chunks_per_dma
    A8 = 2 * A
    G = 64 // A
    G8 = 64 // A8
    C = Uc
    NB = 512
    n_blocks = (C + NB - 1) // NB

    nc = bass.Bass("TRN2", target_bir_lowering=False, debug=False,
                   num_devices=N_CORES)
    wh = nc.dram_tensor("wh", [G, 128, A * C], mybir.dt.float16,
                        kind="ExternalInput").ap()
    wl = nc.dram_tensor("wl", [G8, 128, A8 * C], mybir.dt.float8e4,
                        kind="ExternalInput").ap()
    x2h = nc.dram_tensor("x2h", [128, 64 * 64], mybir.dt.float16,
                         kind="ExternalInput").ap()
    x2l = nc.dram_tensor("x2l", [128, 64 * 32], mybir.dt.float8e4,
                         kind="ExternalInput").ap()
    f = nc.dram_tensor("f", [32, Uc], mybir.dt.float32,
                       kind="ExternalOutput").ap()

    with tile.TileContext(nc) as tc:
        with (
            tc.tile_pool(name="x2p", bufs=1) as x2p,
            tc.tile_pool(name="wp", bufs=wp_bufs) as wp,
            tc.tile_pool(name="wp8", bufs=2) as wp8,
            tc.tile_pool(name="psum", bufs=1, space="PSUM") as pp,
            tc.tile_pool(name="outp", bufs=1) as outp,
        ):
            x2h_t = x2p.tile([128, 64 * 64], mybir.dt.float16, name="x2h_t")
            nc.sync.dma_start(x2h_t[:], x2h[:])
            x2l_t = x2p.tile([128, 64 * 32], mybir.dt.float8e4, name="x2l_t")
            nc.sync.dma_start(x2l_t[:], x2l[:])

            psH = [pp.tile([64, min(NB, C - nb * NB)], mybir.dt.float32,
                           name=f"psH{nb}") for nb in range(n_blocks)]
            psL = [pp.tile([32, min(NB, C - nb * NB)], mybir.dt.float32,
                           name=f"psL{nb}") for nb in range(n_blocks)]
            out_t = outp.tile([32, Uc], mybir.dt.float32)

            # Interleave hi (ACT queue) and lo (SP queue) DMA groups.
            # hi group g covers chunks [g*A, (g+1)*A); lo group covers 2A.
            for g in range(G):
                wh_t = wp.tile([128, A * C], mybir.dt.float16, tag="wh")
                nc.scalar.dma_start(wh_t[:], wh[g, :, :])
                if g % 2 == 0:
                    g8 = g // 2
                    wl_t = wp8.tile([128, A8 * C], mybir.dt.float8e4,
                                    tag="wl")
                    nc.sync.dma_start(wl_t[:], wl[g8, :, :])
                for a in range(A):
                    kb = g * A + a
                    lhsT = x2h_t[:, kb * 64 : (kb + 1) * 64]
                    for nb in range(n_blocks):
                        w_nb = min(NB, C - nb * NB)
                        rhs = wh_t[:, a * C + nb * NB : a * C + nb * NB + w_nb]
                        nc.tensor.matmul(psH[nb][:, :], lhsT, rhs,
                                         start=(kb == 0), stop=(kb == 63))
                if g % 2 == 1:
                    # wl_t holds chunks [ (g//2)*2A, (g//2+1)*2A )
                    for a8 in range(A8):
                        kb = (g // 2) * A8 + a8
                        lhsT8 = x2l_t[:, kb * 32 : (kb + 1) * 32]
                        for nb in range(n_blocks):
                            w_nb = min(NB, C - nb * NB)
                            rhs = wl_t[:, a8 * C + nb * NB
                                       : a8 * C + nb * NB + w_nb]
                            nc.tensor.matmul(psL[nb][:, :], lhsT8, rhs,
                                             start=(kb == 0),
                                             stop=(kb == 63))

            for nb in range(n_blocks):
                w_nb = min(NB, C - nb * NB)
                sl = slice(nb * NB, nb * NB + w_nb)
                nc.vector.tensor_copy(out_t[:, sl], psH[nb][0:32, :])
                nc.vector.tensor_add(out_t[:, sl], out_t[:, sl],
                                     psH[nb][32:64, :])
                lo_sc = outp.tile([32, w_nb], mybir.dt.float32, tag="losc")
                nc.scalar.activation(
                    lo_sc[:, :], psL[nb][:, :],
                    mybir.ActivationFunctionType.Copy,
                    scale=1.0 / MIX3_SCALE,
                )
                nc.vector.tensor_add(out_t[:, sl], out_t[:, sl], lo_sc[:, :])
            nc.sync.dma_start(f[:], out_t[:])
    _split_waits(nc)
    return nc


_NC_CACHE = {}


def _get_nc(Uc, mode, chunks_per_dma):
    key = (Uc, mode, chunks_per_dma)
    if key not in _NC_CACHE:
        _NC_CACHE[key] = _build_nc(Uc, mode, chunks_per_dma)
    return _NC_CACHE[key]


# ---------------------------------------------------------------------------
# Host side


def _split_hilo(arr_f32):
    hi = arr_f32.astype(BF16)
    lo = (arr_f32 - hi.astype(np.float32)).astype(BF16)
    return hi, lo


def _prepare(x, weight, node_in, top, bottom, left, right, mode, A):
    """Host prep: dedup indices, build per-core input maps. Returns
    (in_maps, meta)."""
    T, three, Mdim = x.shape
    assert three == 3
    N, K4, two = weight.shape
    K = K4  # 4*M

    # ---- dedup + shard -----------------------------------------------------
    idx_all = np.concatenate([node_in, top, bottom, left, right]) - 1
    uniq, inv = np.unique(idx_all, return_inverse=True)
    U0 = len(uniq)
    Uc = -(-U0 // N_CORES)
    Uc = ((Uc + 7) // 8) * 8  # pad to multiple of 8
    Upad = Uc * N_CORES
    uniq_pad = np.zeros(Upad, dtype=np.int64)
    uniq_pad[:U0] = uniq

    # ---- x_out and stationary operand -------------------------------------
    xo = np.stack([x[:, 0, :], x[:, 2, :], x[:, 2, :], x[:, 1, :]],
                  axis=-1).reshape(T, K)  # [T, 4M]
    K2 = 2 * K  # interleaved contraction k' = 2i + o
    K_CHUNKS = K2 // 128
    assert K2 == K_CHUNKS * 128

    def _stationary(parts, np_dt):
        """Build [128, 64 * 16*len(parts)*2] chunk-grouped stationary from a
        list of [T, K] matrices (each gets o=0/o=1 zero-interleaved cols)."""
        M = 2 * T * len(parts)
        X2 = np.zeros((K2, M), dtype=np_dt)
        for h, p in enumerate(parts):
            X2[0::2, 2 * h * T : (2 * h + 1) * T] = p.T
            X2[1::2, (2 * h + 1) * T : (2 * h + 2) * T] = p.T
        return np.ascontiguousarray(
            X2.reshape(K_CHUNKS, 128, M).transpose(1, 0, 2).reshape(
                128, K_CHUNKS * M)
        )

    FP8 = ml_dtypes.float8_e4m3
    if mode == "hilo":
        x_hi, x_lo = _split_hilo(xo)
        x2r = _stationary([x_hi, x_lo], BF16)
    elif mode == "mix3":
        xh16 = xo.astype(np.float16)
        xl16 = (xo - xh16.astype(np.float32)).astype(np.float16)
        x2r_h = _stationary([xh16, xl16], np.float16)
        x2r_l = _stationary([xo.astype(FP8)], FP8)
    elif mode == "fp16":
        x2r = _stationary([xo.astype(np.float16)], np.float16)
    else:
        x2r = _stationary([xo], np.float32)

    # ---- per-core weight shards -------------------------------------------
    def _grouped(V_T_src, A_):
        """[Uc, Kt] value matrix -> chunk-grouped [G, 128, A_*Uc]."""
        Kt = V_T_src.shape[1]
        G_ = Kt // 128 // A_
        return np.ascontiguousarray(
            V_T_src.T.reshape(G_, A_, 128, Uc).transpose(0, 2, 1, 3).reshape(
                G_, 128, A_ * Uc
            )
        )

    wf = weight.reshape(N, K2)  # row n: k' = 2i+o contiguous
    in_maps = []
    for c in range(N_CORES):
        rows = uniq_pad[c * Uc : (c + 1) * Uc]
        Wg = wf[rows]  # [Uc, K2] f32
        if mode == "hilo":
            hi, lo = _split_hilo(Wg)
            V = np.concatenate([hi, lo], axis=1)  # [Uc, 2*K2]
            in_maps.append({"w": _grouped(V, A), "x2": x2r})
        elif mode == "mix3":
            wh16 = Wg.astype(np.float16)
            wl8 = ((Wg - wh16.astype(np.float32)) * MIX3_SCALE).astype(FP8)
            in_maps.append({
                "wh": _grouped(wh16, A),
                "wl": _grouped(wl8, 2 * A),
                "x2h": x2r_h,
                "x2l": x2r_l,
            })
        elif mode == "fp16":
            in_maps.append({"w": _grouped(Wg.astype(np.float16), A),
                            "x2": x2r})
        else:
            in_maps.append({"w": _grouped(Wg, A), "x2": x2r})

    meta = dict(T=T, Uc=Uc, Upad=Upad, inv=inv, mode=mode, A=A)
    return in_maps, meta


def _assemble(per_core_f, meta, node_in, top):
    """Unshard: build f_uniq, gather f_in, boundary sums."""
    T, Uc, Upad, inv = meta["T"], meta["Uc"], meta["Upad"], meta["inv"]
    n_in = node_in.shape[0]
    n_b = top.shape[0]
    f_uniq = np.empty((Upad, T, 2), dtype=np.float32)
    for c in range(N_CORES):
        fc = per_core_f[c]  # [32, Uc]: rows o*T+t
        f_uniq[c * Uc : (c + 1) * Uc] = fc.reshape(2, T, Uc).transpose(2, 1, 0)

    f_in = np.ascontiguousarray(f_uniq[inv[:n_in]])
    bi = inv[n_in:]
    s_top = f_uniq[bi[0:n_b], :, 1].sum(axis=0)
    s_bot = f_uniq[bi[n_b : 2 * n_b], :, 1].sum(axis=0)
    s_left = f_uniq[bi[2 * n_b : 3 * n_b], :, 0].sum(axis=0)
    s_right = f_uniq[bi[3 * n_b : 4 * n_b], :, 0].sum(axis=0)
    f_b = np.stack(
        [s_top, s_bot, s_left, s_right, s_top + s_bot, s_left + s_right],
        axis=0,
    )[..., None].astype(np.float32)
    return f_in, f_b


def kernel(x, weight, node_in, top, bottom, left, right):
    global LAST_EXEC_NS, LAST_RESULTS
    x = np.asarray(x, dtype=np.float32)
    weight = np.asarray(weight, dtype=np.float32)
    node_in = np.asarray(node_in).astype(np.int64)
    top = np.asarray(top).astype(np.int64)
    bottom = np.asarray(bottom).astype(np.int64)
    left = np.asarray(left).astype(np.int64)
    right = np.asarray(right).astype(np.int64)

    in_maps, meta = _prepare(x, weight, node_in, top, bottom, left, right,
                             MODE, CHUNKS_PER_DMA)
    nc = _get_nc(meta["Uc"], MODE, CHUNKS_PER_DMA)
    if TRACE:
        _install_ntff_hook()
        import tempfile

        res = run_bass_kernel_spmd(
            nc, in_maps, list(range(N_CORES)), trace=True,
            tmpdir=tempfile.mkdtemp(prefix="eq_trace_"),
        )
        LAST_EXEC_NS = res.exec_time_ns
    else:
        res = run_bass_kernel_spmd(nc, in_maps, list(range(N_CORES)))
    LAST_RESULTS = res

    per_core_f = [res.results[c]["f"] for c in range(N_CORES)]
    return _assemble(per_core_f, meta, node_in, top)


# revision 28
# speedup vs baseline: 1.0230x; 1.0230x over previous
"""Trainium2 Bass kernel for nn_Equilibrium (scatter_memory).

Computation (reference):
    x_out[t, 4m+0..3] = [x[t,0,m], x[t,2,m], x[t,2,m], x[t,1,m]]   # [T, 4M]
    f = einsum('ti,nio->nto', x_out, weight)                       # [N, T, 2]
    f_in = f[node_in - 1]                                          # [N_in, T, 2]
    f_b  = boundary sums over top/bottom/left/right                # [6, T, 1]

Strategy: only nodes referenced by node_in/top/bottom/left/right need f.
Host dedupes those indices (~5.7k unique of 10k), shards the unique set
across 8 cores, and pre-lays-out each core's weight rows as a transposed
[k', n] matrix so the contraction dim sits on SBUF partitions with fully
contiguous DMA.  The device streams its weight shard (memory-bound) through
TensorE:  f32 values are split into bf16 hi+lo parts on host; the PE
computes all four cross products at bf16 rate into fp32 PSUM, giving
~fp32 accuracy.  Host reassembles f_in (gather) and the 6 tiny boundary
sums from the per-core outputs.
"""

import os
import sys
import types

import numpy as np
import ml_dtypes

import concourse.bass as bass
import concourse.mybir as mybir
import concourse.tile as tile
from concourse.vector_clock import ScopedClock
from concourse.bass_utils import run_bass_kernel_spmd

N_CORES = 8
BF16 = ml_dtypes.bfloat16

# mode: "hilo" (bf16 hi/lo split, ~fp32 accurate), "f32", "f32r"
MODE = os.environ.get("EQ_KERNEL_MODE", "hilo")
CHUNKS_PER_DMA = int(os.environ.get("EQ_CHUNKS_PER_DMA", "8"))
TRACE = bool(int(os.environ.get("EQ_KERNEL_TRACE", "0")))
WP_BUFS = int(os.environ.get("EQ_WP_BUFS", "4"))
LAST_EXEC_NS = None
LAST_RESULTS = None

# ---------------------------------------------------------------------------
# Patch 1: the Tile tail drain may carry more sem waits than walrus's
# per-instruction cap (1 for CTRL/Drain in this toolchain). Split them.
_DRAIN_MAX_WAITS = 1


def _patched_drain_and_barrier(self, tick_clock, wait_clock):
    nc = self.nc
    drain_inst = nc.sync.drain()
    wait_clock.add_sem_waits(
        drain_inst.ins, ScopedClock({None: tick_clock.global_clock})
    )
    si = drain_inst.ins.sync_info
    if si is not None and len(si.on_wait) > _DRAIN_MAX_WAITS:
        waits = list(si.on_wait)
        si.on_wait = waits[:_DRAIN_MAX_WAITS]
        for i in range(_DRAIN_MAX_WAITS, len(waits), _DRAIN_MAX_WAITS):
            extra = nc.sync.drain()
            extra.ins.sync_info = mybir.SyncInfo(
                on_wait=list(waits[i : i + _DRAIN_MAX_WAITS]), on_update=[]
            )
    nc.all_engine_barrier()
    assert self.sems is not None
    popped = nc._tile_sem_poison_stack.pop()
    assert popped is self._sem_poison
    nc.clear_and_free_semaphores(list(self.sems.allocated().values()))
    nc.all_engine_barrier()


tile.TileContext._drain_and_barrier = _patched_drain_and_barrier


def _split_waits(nc, cap=_DRAIN_MAX_WAITS):
    """Walrus in this toolchain accepts at most one sem wait per
    instruction; hoist extra waits onto preceding same-engine NoOps."""
    n_id = 0
    for fn in nc.m.functions:
        for bb in fn.blocks:
            insts = list(bb.instructions)
            out = []
            changed = False
            for inst in insts:
                si = inst.sync_info
                if si is not None and len(si.on_wait) > cap:
                    waits = list(si.on_wait)
                    n_extra = len(waits) - cap
                    for i in range(0, n_extra, cap):
                        nop = mybir.InstNoOp(
                            name=f"waitsplit_{n_id}",
                            engine=inst.engine,
                            bass_nofuse=True,
                            sync_info=mybir.SyncInfo(
                                on_wait=list(waits[i : i + cap]), on_update=[]
                            ),
                        )
                        n_id += 1
                        nc.register_instruction(nop)
                        out.append(nop)
                    si.on_wait = waits[n_extra:]
                    changed = True
                out.append(inst)
            if changed:
                bb.instructions = out


# Patch 2: NTFF profile hook (only needed when tracing; the image's antenv
# lacks axon_hooks, so register a ctypes-based hook ourselves).
def _install_ntff_hook():
    try:
        from antenv.axon_hooks import get_axon_ntff_profile_hook  # noqa: F401

        return
    except ImportError:
        pass
    try:
        from trn_agent_boot.trn_boot import _ntff_profile_via_ctypes

        hook = _ntff_profile_via_ctypes("/opt/axon/libaxon_pjrt.so")
    except Exception:
        hook = None
    import antenv

    mod = types.ModuleType("antenv.axon_hooks")
    mod.get_axon_ntff_profile_hook = lambda: hook
    sys.modules["antenv.axon_hooks"] = mod
    antenv.axon_hooks = mod


# ---------------------------------------------------------------------------
# Device kernel builder


def _build_nc(Uc, mode, chunks_per_dma, wp_bufs=WP_BUFS):
    """Build the per-core Bass program.

    hilo mode stacks W_hi then W_lo along the contraction axis (128
    k-chunks; the stationary x2 = [x_hi | x_lo] repeats for both halves),
    so PSUM accumulates all four bf16 cross products: psum[0:32] =
    x_hi@(W_hi+W_lo), psum[32:64] = x_lo@(W_hi+W_lo); one DVE add
    finishes the job.

    Inputs (per core):
      w  : [G, 128, A*Uc]  moving operand (weight shard, transposed)
           G*A = n_chunks k-chunks of 128 rows; Uc columns each.
      x2 : [128, 64*M]     stationary operand; chunk kb uses cols (kb%64)*M.
    Output:
      f  : [32, Uc] f32    f[o*16+t, u]
    """
    A = chunks_per_dma
    if mode == "hilo":
        M = 64
        n_chunks = 128
        dt_in = mybir.dt.bfloat16
    elif mode == "f32":
        M = 32
        n_chunks = 64
        dt_in = mybir.dt.float32
    elif mode == "f32r":
        M = 32
        n_chunks = 64
        dt_in = mybir.dt.float32r
    elif mode == "fp16":
        M = 32
        n_chunks = 64
        dt_in = mybir.dt.float16
    elif mode == "mix3":
        return _build_nc_mix3(Uc, chunks_per_dma, wp_bufs)
    else:
        raise ValueError(mode)
    G = n_chunks // A
    C = Uc
    NB = 512
    n_blocks = (C + NB - 1) // NB

    nc = bass.Bass("TRN2", target_bir_lowering=False, debug=False,
                   num_devices=N_CORES)
    w = nc.dram_tensor("w", [G, 128, A * C], dt_in, kind="ExternalInput").ap()
    x2 = nc.dram_tensor("x2", [128, 64 * M], dt_in,
                        kind="ExternalInput").ap()
    f = nc.dram_tensor("f", [32, Uc], mybir.dt.float32,
                       kind="ExternalOutput").ap()

    with tile.TileContext(nc) as tc:
        with (
            tc.tile_pool(name="x2p", bufs=1) as x2p,
            tc.tile_pool(name="wp", bufs=wp_bufs) as wp,
            tc.tile_pool(name="psum", bufs=1, space="PSUM") as pp,
            tc.tile_pool(name="outp", bufs=1) as outp,
        ):
            x2_t = x2p.tile([128, 64 * M], dt_in)
            nc.sync.dma_start(x2_t[:], x2[:])

            psums = [pp.tile([M, min(NB, C - nb * NB)], mybir.dt.float32,
                             name=f"ps{nb}")
                     for nb in range(n_blocks)]
            out_t = outp.tile([32, Uc], mybir.dt.float32)

            for g in range(G):
                w_t = wp.tile([128, A * C], dt_in, tag="w")
                # alternate the two HWDGE queues (ACT / SP); w0 goes on ACT
                # so it transfers concurrently with x2 (which is on SP).
                eng = nc.scalar if g % 2 == 0 else nc.sync
                eng.dma_start(w_t[:], w[g, :, :])
                for a in range(A):
                    kb = g * A + a
                    j = kb % 64
                    lhsT = x2_t[:, j * M : (j + 1) * M]
                    for nb in range(n_blocks):
                        w_nb = min(NB, C - nb * NB)
                        rhs = w_t[:, a * C + nb * NB : a * C + nb * NB + w_nb]
                        nc.tensor.matmul(
                            psums[nb][:, :],
                            lhsT,
                            rhs,
                            start=(kb == 0),
                            stop=(kb == n_chunks - 1),
                        )

            # Combine psum blocks into f32 output [32, Uc].
            for nb in range(n_blocks):
                w_nb = min(NB, C - nb * NB)
                sl = slice(nb * NB, nb * NB + w_nb)
                if mode == "hilo":
                    nc.vector.tensor_copy(out_t[:, sl], psums[nb][0:32, :])
                    nc.vector.tensor_add(out_t[:, sl], out_t[:, sl],
                                         psums[nb][32:64, :])
                else:
                    nc.vector.tensor_copy(out_t[:, sl], psums[nb][:, :])
            nc.sync.dma_start(f[:], out_t[:])
    _split_waits(nc)
    return nc


MIX3_SCALE = 2.0 ** 18


def _build_nc_mix3(Uc, chunks_per_dma, wp_bufs):
    """3-byte mixed precision: W = fp16(W) + 2^-18 * fp8e4m3(scaled resid).

    hi half: 64 fp16 chunks, stationary [x_hi | x_lo] fp16 (M=64).
    lo half: 64 fp8e4m3 chunks, stationary x fp8e4m3 (M=32), psum scaled
    by 2^-18 during the combine.
    """
    A = chunks_per_dma
    A8 = 2 * A
    G = 64 // A
    G8 = 64 // A8
    C = Uc
    NB = 512
    n_blocks = (C + NB - 1) // NB

    nc = bass.Bass("TRN2", target_bir_lowering=False, debug=False,
                   num_devices=N_CORES)
    wh = nc.dram_tensor("wh", [G, 128, A * C], mybir.dt.float16,
                        kind="ExternalInput").ap()
    wl = nc.dram_tensor("wl", [G8, 128, A8 * C], mybir.dt.float8e4,
                        kind="ExternalInput").ap()
    x2h = nc.dram_tensor("x2h", [128, 64 * 64], mybir.dt.float16,
                         kind="ExternalInput").ap()
    x2l = nc.dram_tensor("x2l", [128, 64 * 32], mybir.dt.float8e4,
                         kind="ExternalInput").ap()
    f = nc.dram_tensor("f", [32, Uc], mybir.dt.float32,
                       kind="ExternalOutput").ap()

    with tile.TileContext(nc) as tc:
        with (
            tc.tile_pool(name="x2p", bufs=1) as x2p,
            tc.tile_pool(name="wp", bufs=wp_bufs) as wp,
            tc.tile_pool(name="wp8", bufs=2) as wp8,
            tc.tile_pool(name="psum", bufs=1, space="PSUM") as pp,
            tc.tile_pool(name="outp", bufs=1) as outp,
        ):
            x2h_t = x2p.tile([128, 64 * 64], mybir.dt.float16, name="x2h_t")
            nc.sync.dma_start(x2h_t[:], x2h[:])
            x2l_t = x2p.tile([128, 64 * 32], mybir.dt.float8e4, name="x2l_t")
            nc.sync.dma_start(x2l_t[:], x2l[:])

            psH = [pp.tile([64, min(NB, C - nb * NB)], mybir.dt.float32,
                           name=f"psH{nb}") for nb in range(n_blocks)]
            psL = [pp.tile([32, min(NB, C - nb * NB)], mybir.dt.float32,
                           name=f"psL{nb}") for nb in range(n_blocks)]
            out_t = outp.tile([32, Uc], mybir.dt.float32)

            # Interleave hi (ACT queue) and lo (SP queue) DMA groups.
            # hi group g covers chunks [g*A, (g+1)*A); lo group covers 2A.
            for g in range(G):
                wh_t = wp.tile([128, A * C], mybir.dt.float16, tag="wh")
                heng = nc.scalar if g % 2 == 0 else nc.sync
                heng.dma_start(wh_t[:], wh[g, :, :])
                if g % 2 == 0:
                    g8 = g // 2
                    wl_t = wp8.tile([128, A8 * C], mybir.dt.float8e4,
                                    tag="wl")
                    leng = nc.sync if g % 2 == 0 else nc.scalar
                    leng.dma_start(wl_t[:], wl[g8, :, :])
                for a in range(A):
                    kb = g * A + a
                    lhsT = x2h_t[:, kb * 64 : (kb + 1) * 64]
                    for nb in range(n_blocks):
                        w_nb = min(NB, C - nb * NB)
                        rhs = wh_t[:, a * C + nb * NB : a * C + nb * NB + w_nb]
                        nc.tensor.matmul(psH[nb][:, :], lhsT, rhs,
                                         start=(kb == 0), stop=(kb == 63))
                if g % 2 == 1:
                    # wl_t holds chunks [ (g//2)*2A, (g//2+1)*2A )
                    for a8 in range(A8):
                        kb = (g // 2) * A8 + a8
                        lhsT8 = x2l_t[:, kb * 32 : (kb + 1) * 32]
                        for nb in range(n_blocks):
                            w_nb = min(NB, C - nb * NB)
                            rhs = wl_t[:, a8 * C + nb * NB
                                       : a8 * C + nb * NB + w_nb]
                            nc.tensor.matmul(psL[nb][:, :], lhsT8, rhs,
                                             start=(kb == 0),
                                             stop=(kb == 63))

            for nb in range(n_blocks):
                w_nb = min(NB, C - nb * NB)
                sl = slice(nb * NB, nb * NB + w_nb)
                nc.vector.tensor_copy(out_t[:, sl], psH[nb][0:32, :])
                nc.vector.tensor_add(out_t[:, sl], out_t[:, sl],
                                     psH[nb][32:64, :])
                lo_sc = outp.tile([32, w_nb], mybir.dt.float32, tag="losc")
                nc.scalar.activation(
                    lo_sc[:, :], psL[nb][:, :],
                    mybir.ActivationFunctionType.Copy,
                    scale=1.0 / MIX3_SCALE,
                )
                nc.vector.tensor_add(out_t[:, sl], out_t[:, sl], lo_sc[:, :])
            nc.sync.dma_start(f[:], out_t[:])
    _split_waits(nc)
    return nc


_NC_CACHE = {}


def _get_nc(Uc, mode, chunks_per_dma):
    key = (Uc, mode, chunks_per_dma)
    if key not in _NC_CACHE:
        _NC_CACHE[key] = _build_nc(Uc, mode, chunks_per_dma)
    return _NC_CACHE[key]


# ---------------------------------------------------------------------------
# Host side


def _split_hilo(arr_f32):
    hi = arr_f32.astype(BF16)
    lo = (arr_f32 - hi.astype(np.float32)).astype(BF16)
    return hi, lo


def _prepare(x, weight, node_in, top, bottom, left, right, mode, A):
    """Host prep: dedup indices, build per-core input maps. Returns
    (in_maps, meta)."""
    T, three, Mdim = x.shape
    assert three == 3
    N, K4, two = weight.shape
    K = K4  # 4*M

    # ---- dedup + shard -----------------------------------------------------
    idx_all = np.concatenate([node_in, top, bottom, left, right]) - 1
    uniq, inv = np.unique(idx_all, return_inverse=True)
    U0 = len(uniq)
    Uc = -(-U0 // N_CORES)
    Uc = ((Uc + 7) // 8) * 8  # pad to multiple of 8
    Upad = Uc * N_CORES
    uniq_pad = np.zeros(Upad, dtype=np.int64)
    uniq_pad[:U0] = uniq

    # ---- x_out and stationary operand -------------------------------------
    xo = np.stack([x[:, 0, :], x[:, 2, :], x[:, 2, :], x[:, 1, :]],
                  axis=-1).reshape(T, K)  # [T, 4M]
    K2 = 2 * K  # interleaved contraction k' = 2i + o
    K_CHUNKS = K2 // 128
    assert K2 == K_CHUNKS * 128

    def _stationary(parts, np_dt):
        """Build [128, 64 * 16*len(parts)*2] chunk-grouped stationary from a
        list of [T, K] matrices (each gets o=0/o=1 zero-interleaved cols)."""
        M = 2 * T * len(parts)
        X2 = np.zeros((K2, M), dtype=np_dt)
        for h, p in enumerate(parts):
            X2[0::2, 2 * h * T : (2 * h + 1) * T] = p.T
            X2[1::2, (2 * h + 1) * T : (2 * h + 2) * T] = p.T
        return np.ascontiguousarray(
            X2.reshape(K_CHUNKS, 128, M).transpose(1, 0, 2).reshape(
                128, K_CHUNKS * M)
        )

    FP8 = ml_dtypes.float8_e4m3
    if mode == "hilo":
        x_hi, x_lo = _split_hilo(xo)
        x2r = _stationary([x_hi, x_lo], BF16)
    elif mode == "mix3":
        xh16 = xo.astype(np.float16)
        xl16 = (xo - xh16.astype(np.float32)).astype(np.float16)
        x2r_h = _stationary([xh16, xl16], np.float16)
        x2r_l = _stationary([xo.astype(FP8)], FP8)
    elif mode == "fp16":
        x2r = _stationary([xo.astype(np.float16)], np.float16)
    else:
        x2r = _stationary([xo], np.float32)

    # ---- per-core weight shards -------------------------------------------
    def _grouped(V_T_src, A_):
        """[Uc, Kt] value matrix -> chunk-grouped [G, 128, A_*Uc]."""
        Kt = V_T_src.shape[1]
        G_ = Kt // 128 // A_
        return np.ascontiguousarray(
            V_T_src.T.reshape(G_, A_, 128, Uc).transpose(0, 2, 1, 3).reshape(
                G_, 128, A_ * Uc
            )
        )

    wf = weight.reshape(N, K2)  # row n: k' = 2i+o contiguous
    in_maps = []
    for c in range(N_CORES):
        rows = uniq_pad[c * Uc : (c + 1) * Uc]
        Wg = wf[rows]  # [Uc, K2] f32
        if mode == "hilo":
            hi, lo = _split_hilo(Wg)
            V = np.concatenate([hi, lo], axis=1)  # [Uc, 2*K2]
            in_maps.append({"w": _grouped(V, A), "x2": x2r})
        elif mode == "mix3":
            wh16 = Wg.astype(np.float16)
            wl8 = ((Wg - wh16.astype(np.float32)) * MIX3_SCALE).astype(FP8)
            in_maps.append({
                "wh": _grouped(wh16, A),
                "wl": _grouped(wl8, 2 * A),
                "x2h": x2r_h,
                "x2l": x2r_l,
            })
        elif mode == "fp16":
            in_maps.append({"w": _grouped(Wg.astype(np.float16), A),
                            "x2": x2r})
        else:
            in_maps.append({"w": _grouped(Wg, A), "x2": x2r})

    meta = dict(T=T, Uc=Uc, Upad=Upad, inv=inv, mode=mode, A=A)
    return in_maps, meta


def _assemble(per_core_f, meta, node_in, top):
    """Unshard: build f_uniq, gather f_in, boundary sums."""
    T, Uc, Upad, inv = meta["T"], meta["Uc"], meta["Upad"], meta["inv"]
    n_in = node_in.shape[0]
    n_b = top.shape[0]
    f_uniq = np.empty((Upad, T, 2), dtype=np.float32)
    for c in range(N_CORES):
        fc = per_core_f[c]  # [32, Uc]: rows o*T+t
        f_uniq[c * Uc : (c + 1) * Uc] = fc.reshape(2, T, Uc).transpose(2, 1, 0)

    f_in = np.ascontiguousarray(f_uniq[inv[:n_in]])
    bi = inv[n_in:]
    s_top = f_uniq[bi[0:n_b], :, 1].sum(axis=0)
    s_bot = f_uniq[bi[n_b : 2 * n_b], :, 1].sum(axis=0)
    s_left = f_uniq[bi[2 * n_b : 3 * n_b], :, 0].sum(axis=0)
    s_right = f_uniq[bi[3 * n_b : 4 * n_b], :, 0].sum(axis=0)
    f_b = np.stack(
        [s_top, s_bot, s_left, s_right, s_top + s_bot, s_left + s_right],
        axis=0,
    )[..., None].astype(np.float32)
    return f_in, f_b


def kernel(x, weight, node_in, top, bottom, left, right):
    global LAST_EXEC_NS, LAST_RESULTS
    x = np.asarray(x, dtype=np.float32)
    weight = np.asarray(weight, dtype=np.float32)
    node_in = np.asarray(node_in).astype(np.int64)
    top = np.asarray(top).astype(np.int64)
    bottom = np.asarray(bottom).astype(np.int64)
    left = np.asarray(left).astype(np.int64)
    right = np.asarray(right).astype(np.int64)

    in_maps, meta = _prepare(x, weight, node_in, top, bottom, left, right,
                             MODE, CHUNKS_PER_DMA)
    nc = _get_nc(meta["Uc"], MODE, CHUNKS_PER_DMA)
    if TRACE:
        _install_ntff_hook()
        import tempfile

        res = run_bass_kernel_spmd(
            nc, in_maps, list(range(N_CORES)), trace=True,
            tmpdir=tempfile.mkdtemp(prefix="eq_trace_"),
        )
        LAST_EXEC_NS = res.exec_time_ns
    else:
        res = run_bass_kernel_spmd(nc, in_maps, list(range(N_CORES)))
    LAST_RESULTS = res

    per_core_f = [res.results[c]["f"] for c in range(N_CORES)]
    return _assemble(per_core_f, meta, node_in, top)


# revision 30
# speedup vs baseline: 1.1952x; 1.1683x over previous
"""Trainium2 Bass kernel for nn_Equilibrium (scatter_memory).

Computation (reference):
    x_out[t, 4m+0..3] = [x[t,0,m], x[t,2,m], x[t,2,m], x[t,1,m]]   # [T, 4M]
    f = einsum('ti,nio->nto', x_out, weight)                       # [N, T, 2]
    f_in = f[node_in - 1]                                          # [N_in, T, 2]
    f_b  = boundary sums over top/bottom/left/right                # [6, T, 1]

Strategy: only nodes referenced by node_in/top/bottom/left/right need f.
Host dedupes those indices (~5.7k unique of 10k), shards the unique set
across 8 cores, and pre-lays-out each core's weight rows as a transposed
[k', n] matrix so the contraction dim sits on SBUF partitions with fully
contiguous DMA.  The device streams its weight shard (memory-bound) through
TensorE:  f32 values are split into bf16 hi+lo parts on host; the PE
computes all four cross products at bf16 rate into fp32 PSUM, giving
~fp32 accuracy.  Host reassembles f_in (gather) and the 6 tiny boundary
sums from the per-core outputs.
"""

import os
import sys
import types

import numpy as np
import ml_dtypes

import concourse.bass as bass
import concourse.mybir as mybir
import concourse.tile as tile
from concourse.vector_clock import ScopedClock
from concourse.bass_utils import run_bass_kernel_spmd

N_CORES = 8
BF16 = ml_dtypes.bfloat16

# mode: "hilo" (bf16 hi/lo split, ~fp32 accurate), "f32", "f32r"
MODE = os.environ.get("EQ_KERNEL_MODE", "hilo")
CHUNKS_PER_DMA = int(os.environ.get("EQ_CHUNKS_PER_DMA", "8"))
TRACE = bool(int(os.environ.get("EQ_KERNEL_TRACE", "0")))
WP_BUFS = int(os.environ.get("EQ_WP_BUFS", "4"))
LAST_EXEC_NS = None
LAST_RESULTS = None

# ---------------------------------------------------------------------------
# Patch 1: the Tile tail drain may carry more sem waits than walrus's
# per-instruction cap (1 for CTRL/Drain in this toolchain). Split them.
_DRAIN_MAX_WAITS = 1


def _patched_drain_and_barrier(self, tick_clock, wait_clock):
    nc = self.nc
    drain_inst = nc.sync.drain()
    wait_clock.add_sem_waits(
        drain_inst.ins, ScopedClock({None: tick_clock.global_clock})
    )
    si = drain_inst.ins.sync_info
    if si is not None and len(si.on_wait) > _DRAIN_MAX_WAITS:
        waits = list(si.on_wait)
        si.on_wait = waits[:_DRAIN_MAX_WAITS]
        for i in range(_DRAIN_MAX_WAITS, len(waits), _DRAIN_MAX_WAITS):
            extra = nc.sync.drain()
            extra.ins.sync_info = mybir.SyncInfo(
                on_wait=list(waits[i : i + _DRAIN_MAX_WAITS]), on_update=[]
            )
    nc.all_engine_barrier()
    assert self.sems is not None
    popped = nc._tile_sem_poison_stack.pop()
    assert popped is self._sem_poison
    nc.clear_and_free_semaphores(list(self.sems.allocated().values()))
    nc.all_engine_barrier()


tile.TileContext._drain_and_barrier = _patched_drain_and_barrier


def _split_waits(nc, cap=_DRAIN_MAX_WAITS):
    """Walrus in this toolchain accepts at most one sem wait per
    instruction; hoist extra waits onto preceding same-engine NoOps."""
    n_id = 0
    for fn in nc.m.functions:
        for bb in fn.blocks:
            insts = list(bb.instructions)
            out = []
            changed = False
            for inst in insts:
                si = inst.sync_info
                if si is not None and len(si.on_wait) > cap:
                    waits = list(si.on_wait)
                    n_extra = len(waits) - cap
                    for i in range(0, n_extra, cap):
                        nop = mybir.InstNoOp(
                            name=f"waitsplit_{n_id}",
                            engine=inst.engine,
                            bass_nofuse=True,
                            sync_info=mybir.SyncInfo(
                                on_wait=list(waits[i : i + cap]), on_update=[]
                            ),
                        )
                        n_id += 1
                        nc.register_instruction(nop)
                        out.append(nop)
                    si.on_wait = waits[n_extra:]
                    changed = True
                out.append(inst)
            if changed:
                bb.instructions = out


# Patch 2: NTFF profile hook (only needed when tracing; the image's antenv
# lacks axon_hooks, so register a ctypes-based hook ourselves).
def _install_ntff_hook():
    try:
        from antenv.axon_hooks import get_axon_ntff_profile_hook  # noqa: F401

        return
    except ImportError:
        pass
    try:
        from trn_agent_boot.trn_boot import _ntff_profile_via_ctypes

        hook = _ntff_profile_via_ctypes("/opt/axon/libaxon_pjrt.so")
    except Exception:
        hook = None
    import antenv

    mod = types.ModuleType("antenv.axon_hooks")
    mod.get_axon_ntff_profile_hook = lambda: hook
    sys.modules["antenv.axon_hooks"] = mod
    antenv.axon_hooks = mod


# ---------------------------------------------------------------------------
# Device kernel builder


def _build_nc(Uc, mode, chunks_per_dma, wp_bufs=WP_BUFS):
    """Build the per-core Bass program.

    hilo mode stacks W_hi then W_lo along the contraction axis (128
    k-chunks; the stationary x2 = [x_hi | x_lo] repeats for both halves),
    so PSUM accumulates all four bf16 cross products: psum[0:32] =
    x_hi@(W_hi+W_lo), psum[32:64] = x_lo@(W_hi+W_lo); one DVE add
    finishes the job.

    Inputs (per core):
      w  : [G, 128, A*Uc]  moving operand (weight shard, transposed)
           G*A = n_chunks k-chunks of 128 rows; Uc columns each.
      x2 : [128, 64*M]     stationary operand; chunk kb uses cols (kb%64)*M.
    Output:
      f  : [32, Uc] f32    f[o*16+t, u]
    """
    A = chunks_per_dma
    if mode == "hilo":
        M = 64
        n_chunks = 128
        dt_in = mybir.dt.bfloat16
    elif mode == "f32":
        M = 32
        n_chunks = 64
        dt_in = mybir.dt.float32
    elif mode == "f32r":
        M = 32
        n_chunks = 64
        dt_in = mybir.dt.float32r
    elif mode == "fp16":
        M = 32
        n_chunks = 64
        dt_in = mybir.dt.float16
    elif mode == "mix3":
        return _build_nc_mix3(Uc, chunks_per_dma, wp_bufs)
    else:
        raise ValueError(mode)
    G = n_chunks // A
    C = Uc
    NB = 512
    n_blocks = (C + NB - 1) // NB

    nc = bass.Bass("TRN2", target_bir_lowering=False, debug=False,
                   num_devices=N_CORES)
    w = nc.dram_tensor("w", [G, 128, A * C], dt_in, kind="ExternalInput").ap()
    x2 = nc.dram_tensor("x2", [128, 64 * M], dt_in,
                        kind="ExternalInput").ap()
    f = nc.dram_tensor("f", [32, Uc], mybir.dt.float32,
                       kind="ExternalOutput").ap()

    with tile.TileContext(nc) as tc:
        with (
            tc.tile_pool(name="x2p", bufs=1) as x2p,
            tc.tile_pool(name="wp", bufs=wp_bufs) as wp,
            tc.tile_pool(name="psum", bufs=1, space="PSUM") as pp,
            tc.tile_pool(name="outp", bufs=1) as outp,
        ):
            x2_t = x2p.tile([128, 64 * M], dt_in)
            nc.sync.dma_start(x2_t[:], x2[:])

            psums = [pp.tile([M, min(NB, C - nb * NB)], mybir.dt.float32,
                             name=f"ps{nb}")
                     for nb in range(n_blocks)]
            out_t = outp.tile([32, Uc], mybir.dt.float32)

            for g in range(G):
                w_t = wp.tile([128, A * C], dt_in, tag="w")
                # alternate the two HWDGE queues (ACT / SP); w0 goes on ACT
                # so it transfers concurrently with x2 (which is on SP).
                eng = nc.scalar if g % 2 == 0 else nc.sync
                eng.dma_start(w_t[:], w[g, :, :])
                for a in range(A):
                    kb = g * A + a
                    j = kb % 64
                    lhsT = x2_t[:, j * M : (j + 1) * M]
                    for nb in range(n_blocks):
                        w_nb = min(NB, C - nb * NB)
                        rhs = w_t[:, a * C + nb * NB : a * C + nb * NB + w_nb]
                        nc.tensor.matmul(
                            psums[nb][:, :],
                            lhsT,
                            rhs,
                            start=(kb == 0),
                            stop=(kb == n_chunks - 1),
                        )

            # Combine psum blocks into f32 output [32, Uc].
            for nb in range(n_blocks):
                w_nb = min(NB, C - nb * NB)
                sl = slice(nb * NB, nb * NB + w_nb)
                if mode == "hilo":
                    nc.vector.tensor_copy(out_t[:, sl], psums[nb][0:32, :])
                    nc.vector.tensor_add(out_t[:, sl], out_t[:, sl],
                                         psums[nb][32:64, :])
                else:
                    nc.vector.tensor_copy(out_t[:, sl], psums[nb][:, :])
            nc.sync.dma_start(f[:], out_t[:])
    _split_waits(nc)
    return nc


MIX3_SCALE = 2.0 ** 18


def _build_nc_mix3(Uc, chunks_per_dma, wp_bufs):
    """3-byte mixed precision: W = fp16(W) + 2^-18 * fp8e4m3(scaled resid).

    hi half: 64 fp16 chunks, stationary [x_hi | x_lo] fp16 (M=64).
    lo half: 64 fp8e4m3 chunks, stationary x fp8e4m3 (M=32), psum scaled
    by 2^-18 during the combine.
    """
    A = chunks_per_dma
    A8 = 2 * A
    G = 64 // A
    G8 = 64 // A8
    C = Uc
    NB = 512
    n_blocks = (C + NB - 1) // NB

    nc = bass.Bass("TRN2", target_bir_lowering=False, debug=False,
                   num_devices=N_CORES)
    wh = nc.dram_tensor("wh", [G, 128, A * C], mybir.dt.float16,
                        kind="ExternalInput").ap()
    wl = nc.dram_tensor("wl", [G8, 128, A8 * C], mybir.dt.float8e4,
                        kind="ExternalInput").ap()
    x2h = nc.dram_tensor("x2h", [128, 64 * 64], mybir.dt.float16,
                         kind="ExternalInput").ap()
    x2l = nc.dram_tensor("x2l", [128, 64 * 32], mybir.dt.float8e4,
                         kind="ExternalInput").ap()
    f = nc.dram_tensor("f", [32, Uc], mybir.dt.float32,
                       kind="ExternalOutput").ap()

    with tile.TileContext(nc) as tc:
        with (
            tc.tile_pool(name="x2p", bufs=1) as x2p,
            tc.tile_pool(name="wp", bufs=wp_bufs) as wp,
            tc.tile_pool(name="wp8", bufs=2) as wp8,
            tc.tile_pool(name="psum", bufs=1, space="PSUM") as pp,
            tc.tile_pool(name="outp", bufs=1) as outp,
        ):
            x2h_t = x2p.tile([128, 64 * 64], mybir.dt.float16, name="x2h_t")
            nc.sync.dma_start(x2h_t[:], x2h[:])
            x2l_t = x2p.tile([128, 64 * 32], mybir.dt.float8e4, name="x2l_t")
            nc.sync.dma_start(x2l_t[:], x2l[:])

            psH = [pp.tile([64, min(NB, C - nb * NB)], mybir.dt.float32,
                           name=f"psH{nb}") for nb in range(n_blocks)]
            psL = [pp.tile([32, min(NB, C - nb * NB)], mybir.dt.float32,
                           name=f"psL{nb}") for nb in range(n_blocks)]
            out_t = outp.tile([32, Uc], mybir.dt.float32)

            # Interleave hi (ACT queue) and lo (SP queue) DMA groups.
            # hi group g covers chunks [g*A, (g+1)*A); lo group covers 2A.
            for g in range(G):
                wh_t = wp.tile([128, A * C], mybir.dt.float16, tag="wh")
                heng = nc.scalar if g % 2 == 0 else nc.sync
                heng.dma_start(wh_t[:], wh[g, :, :])
                if g % 2 == 0:
                    g8 = g // 2
                    wl_t = wp8.tile([128, A8 * C], mybir.dt.float8e4,
                                    tag="wl")
                    leng = nc.sync if g % 2 == 0 else nc.scalar
                    leng.dma_start(wl_t[:], wl[g8, :, :])
                for a in range(A):
                    kb = g * A + a
                    lhsT = x2h_t[:, kb * 64 : (kb + 1) * 64]
                    for nb in range(n_blocks):
                        w_nb = min(NB, C - nb * NB)
                        rhs = wh_t[:, a * C + nb * NB : a * C + nb * NB + w_nb]
                        nc.tensor.matmul(psH[nb][:, :], lhsT, rhs,
                                         start=(kb == 0), stop=(kb == 63))
                if g % 2 == 1:
                    # wl_t holds chunks [ (g//2)*2A, (g//2+1)*2A ).
                    # DoubleRow: process chunk PAIRS (contraction 256) with
                    # 2 fp8 weights per PE cell.
                    n_pairs_per_tile = A8 // 2
                    for a8p in range(n_pairs_per_tile):
                        pair = (g // 2) * n_pairs_per_tile + a8p
                        lhsT8 = x2l_t[
                            :, (2 * pair) * 32 : (2 * pair + 2) * 32
                        ].rearrange("p (c m) -> p c m", c=2)
                        rhs_pair = wl_t[
                            :, (2 * a8p) * C : (2 * a8p + 2) * C
                        ].rearrange("p (c n) -> p c n", c=2)
                        for nb in range(n_blocks):
                            w_nb = min(NB, C - nb * NB)
                            rhs = rhs_pair[:, :, nb * NB : nb * NB + w_nb]
                            nc.tensor.matmul(
                                psL[nb][:, :], lhsT8, rhs,
                                start=(pair == 0), stop=(pair == 31),
                                perf_mode=mybir.MatmulPerfMode.DoubleRow,
                            )

            for nb in range(n_blocks):
                w_nb = min(NB, C - nb * NB)
                sl = slice(nb * NB, nb * NB + w_nb)
                nc.vector.tensor_copy(out_t[:, sl], psH[nb][0:32, :])
                nc.vector.tensor_add(out_t[:, sl], out_t[:, sl],
                                     psH[nb][32:64, :])
                lo_sc = outp.tile([32, w_nb], mybir.dt.float32, tag="losc")
                nc.scalar.activation(
                    lo_sc[:, :], psL[nb][:, :],
                    mybir.ActivationFunctionType.Copy,
                    scale=1.0 / MIX3_SCALE,
                )
                nc.vector.tensor_add(out_t[:, sl], out_t[:, sl], lo_sc[:, :])
            nc.sync.dma_start(f[:], out_t[:])
    _split_waits(nc)
    return nc


_NC_CACHE = {}


def _get_nc(Uc, mode, chunks_per_dma):
    key = (Uc, mode, chunks_per_dma)
    if key not in _NC_CACHE:
        _NC_CACHE[key] = _build_nc(Uc, mode, chunks_per_dma)
    return _NC_CACHE[key]


# ---------------------------------------------------------------------------
# Host side


def _split_hilo(arr_f32):
    hi = arr_f32.astype(BF16)
    lo = (arr_f32 - hi.astype(np.float32)).astype(BF16)
    return hi, lo


def _prepare(x, weight, node_in, top, bottom, left, right, mode, A):
    """Host prep: dedup indices, build per-core input maps. Returns
    (in_maps, meta)."""
    T, three, Mdim = x.shape
    assert three == 3
    N, K4, two = weight.shape
    K = K4  # 4*M

    # ---- dedup + shard -----------------------------------------------------
    idx_all = np.concatenate([node_in, top, bottom, left, right]) - 1
    uniq, inv = np.unique(idx_all, return_inverse=True)
    U0 = len(uniq)
    Uc = -(-U0 // N_CORES)
    Uc = ((Uc + 15) // 16) * 16  # pad (16: DoubleRow AP stride % 16 == 0)
    Upad = Uc * N_CORES
    uniq_pad = np.zeros(Upad, dtype=np.int64)
    uniq_pad[:U0] = uniq

    # ---- x_out and stationary operand -------------------------------------
    xo = np.stack([x[:, 0, :], x[:, 2, :], x[:, 2, :], x[:, 1, :]],
                  axis=-1).reshape(T, K)  # [T, 4M]
    K2 = 2 * K  # interleaved contraction k' = 2i + o
    K_CHUNKS = K2 // 128
    assert K2 == K_CHUNKS * 128

    def _stationary(parts, np_dt):
        """Build [128, 64 * 16*len(parts)*2] chunk-grouped stationary from a
        list of [T, K] matrices (each gets o=0/o=1 zero-interleaved cols)."""
        M = 2 * T * len(parts)
        X2 = np.zeros((K2, M), dtype=np_dt)
        for h, p in enumerate(parts):
            X2[0::2, 2 * h * T : (2 * h + 1) * T] = p.T
            X2[1::2, (2 * h + 1) * T : (2 * h + 2) * T] = p.T
        return np.ascontiguousarray(
            X2.reshape(K_CHUNKS, 128, M).transpose(1, 0, 2).reshape(
                128, K_CHUNKS * M)
        )

    FP8 = ml_dtypes.float8_e4m3
    if mode == "hilo":
        x_hi, x_lo = _split_hilo(xo)
        x2r = _stationary([x_hi, x_lo], BF16)
    elif mode == "mix3":
        xh16 = xo.astype(np.float16)
        xl16 = (xo - xh16.astype(np.float32)).astype(np.float16)
        x2r_h = _stationary([xh16, xl16], np.float16)
        x2r_l = _stationary([xo.astype(FP8)], FP8)
    elif mode == "fp16":
        x2r = _stationary([xo.astype(np.float16)], np.float16)
    else:
        x2r = _stationary([xo], np.float32)

    # ---- per-core weight shards -------------------------------------------
    def _grouped(V_T_src, A_):
        """[Uc, Kt] value matrix -> chunk-grouped [G, 128, A_*Uc]."""
        Kt = V_T_src.shape[1]
        G_ = Kt // 128 // A_
        return np.ascontiguousarray(
            V_T_src.T.reshape(G_, A_, 128, Uc).transpose(0, 2, 1, 3).reshape(
                G_, 128, A_ * Uc
            )
        )

    wf = weight.reshape(N, K2)  # row n: k' = 2i+o contiguous
    in_maps = []
    for c in range(N_CORES):
        rows = uniq_pad[c * Uc : (c + 1) * Uc]
        Wg = wf[rows]  # [Uc, K2] f32
        if mode == "hilo":
            hi, lo = _split_hilo(Wg)
            V = np.concatenate([hi, lo], axis=1)  # [Uc, 2*K2]
            in_maps.append({"w": _grouped(V, A), "x2": x2r})
        elif mode == "mix3":
            wh16 = Wg.astype(np.float16)
            wl8 = ((Wg - wh16.astype(np.float32)) * MIX3_SCALE).astype(FP8)
            in_maps.append({
                "wh": _grouped(wh16, A),
                "wl": _grouped(wl8, 2 * A),
                "x2h": x2r_h,
                "x2l": x2r_l,
            })
        elif mode == "fp16":
            in_maps.append({"w": _grouped(Wg.astype(np.float16), A),
                            "x2": x2r})
        else:
            in_maps.append({"w": _grouped(Wg, A), "x2": x2r})

    meta = dict(T=T, Uc=Uc, Upad=Upad, inv=inv, mode=mode, A=A)
    return in_maps, meta


def _assemble(per_core_f, meta, node_in, top):
    """Unshard: build f_uniq, gather f_in, boundary sums."""
    T, Uc, Upad, inv = meta["T"], meta["Uc"], meta["Upad"], meta["inv"]
    n_in = node_in.shape[0]
    n_b = top.shape[0]
    f_uniq = np.empty((Upad, T, 2), dtype=np.float32)
    for c in range(N_CORES):
        fc = per_core_f[c]  # [32, Uc]: rows o*T+t
        f_uniq[c * Uc : (c + 1) * Uc] = fc.reshape(2, T, Uc).transpose(2, 1, 0)

    f_in = np.ascontiguousarray(f_uniq[inv[:n_in]])
    bi = inv[n_in:]
    s_top = f_uniq[bi[0:n_b], :, 1].sum(axis=0)
    s_bot = f_uniq[bi[n_b : 2 * n_b], :, 1].sum(axis=0)
    s_left = f_uniq[bi[2 * n_b : 3 * n_b], :, 0].sum(axis=0)
    s_right = f_uniq[bi[3 * n_b : 4 * n_b], :, 0].sum(axis=0)
    f_b = np.stack(
        [s_top, s_bot, s_left, s_right, s_top + s_bot, s_left + s_right],
        axis=0,
    )[..., None].astype(np.float32)
    return f_in, f_b


def kernel(x, weight, node_in, top, bottom, left, right):
    global LAST_EXEC_NS, LAST_RESULTS
    x = np.asarray(x, dtype=np.float32)
    weight = np.asarray(weight, dtype=np.float32)
    node_in = np.asarray(node_in).astype(np.int64)
    top = np.asarray(top).astype(np.int64)
    bottom = np.asarray(bottom).astype(np.int64)
    left = np.asarray(left).astype(np.int64)
    right = np.asarray(right).astype(np.int64)

    in_maps, meta = _prepare(x, weight, node_in, top, bottom, left, right,
                             MODE, CHUNKS_PER_DMA)
    nc = _get_nc(meta["Uc"], MODE, CHUNKS_PER_DMA)
    if TRACE:
        _install_ntff_hook()
        import tempfile

        res = run_bass_kernel_spmd(
            nc, in_maps, list(range(N_CORES)), trace=True,
            tmpdir=tempfile.mkdtemp(prefix="eq_trace_"),
        )
        LAST_EXEC_NS = res.exec_time_ns
    else:
        res = run_bass_kernel_spmd(nc, in_maps, list(range(N_CORES)))
    LAST_RESULTS = res

    per_core_f = [res.results[c]["f"] for c in range(N_CORES)]
    return _assemble(per_core_f, meta, node_in, top)


# revision 37
# speedup vs baseline: 1.2360x; 1.0341x over previous
"""Trainium2 Bass kernel for nn_Equilibrium (scatter_memory).

Computation (reference):
    x_out[t, 4m+0..3] = [x[t,0,m], x[t,2,m], x[t,2,m], x[t,1,m]]   # [T, 4M]
    f = einsum('ti,nio->nto', x_out, weight)                       # [N, T, 2]
    f_in = f[node_in - 1]                                          # [N_in, T, 2]
    f_b  = boundary sums over top/bottom/left/right                # [6, T, 1]

Strategy: only nodes referenced by node_in/top/bottom/left/right need f.
Host dedupes those indices (~5.7k unique of 10k), shards the unique set
across 8 cores, and pre-lays-out each core's weight rows as a transposed
[k', n] matrix so the contraction dim sits on SBUF partitions with fully
contiguous DMA.  The device streams its weight shard (memory-bound) through
TensorE:  f32 values are split into bf16 hi+lo parts on host; the PE
computes all four cross products at bf16 rate into fp32 PSUM, giving
~fp32 accuracy.  Host reassembles f_in (gather) and the 6 tiny boundary
sums from the per-core outputs.
"""

import os
import sys
import types

import numpy as np
import ml_dtypes

import concourse.bass as bass
import concourse.mybir as mybir
import concourse.tile as tile
from concourse.vector_clock import ScopedClock
from concourse.bass_utils import run_bass_kernel_spmd

N_CORES = 8
BF16 = ml_dtypes.bfloat16

# mode: "hilo" (bf16 hi/lo split, ~fp32 accurate), "f32", "f32r"
MODE = os.environ.get("EQ_KERNEL_MODE", "hilo")
CHUNKS_PER_DMA = int(os.environ.get("EQ_CHUNKS_PER_DMA", "8"))
TRACE = bool(int(os.environ.get("EQ_KERNEL_TRACE", "0")))
WP_BUFS = int(os.environ.get("EQ_WP_BUFS", "4"))
LAST_EXEC_NS = None
LAST_RESULTS = None

# ---------------------------------------------------------------------------
# Patch 1: the Tile tail drain may carry more sem waits than walrus's
# per-instruction cap (1 for CTRL/Drain in this toolchain). Split them.
_DRAIN_MAX_WAITS = 1


def _patched_drain_and_barrier(self, tick_clock, wait_clock):
    nc = self.nc
    drain_inst = nc.sync.drain()
    wait_clock.add_sem_waits(
        drain_inst.ins, ScopedClock({None: tick_clock.global_clock})
    )
    si = drain_inst.ins.sync_info
    if si is not None and len(si.on_wait) > _DRAIN_MAX_WAITS:
        waits = list(si.on_wait)
        si.on_wait = waits[:_DRAIN_MAX_WAITS]
        for i in range(_DRAIN_MAX_WAITS, len(waits), _DRAIN_MAX_WAITS):
            extra = nc.sync.drain()
            extra.ins.sync_info = mybir.SyncInfo(
                on_wait=list(waits[i : i + _DRAIN_MAX_WAITS]), on_update=[]
            )
    nc.all_engine_barrier()
    assert self.sems is not None
    popped = nc._tile_sem_poison_stack.pop()
    assert popped is self._sem_poison
    nc.clear_and_free_semaphores(list(self.sems.allocated().values()))
    nc.all_engine_barrier()


tile.TileContext._drain_and_barrier = _patched_drain_and_barrier


def _split_waits(nc, cap=_DRAIN_MAX_WAITS):
    """Walrus in this toolchain accepts at most one sem wait per
    instruction; hoist extra waits onto preceding same-engine NoOps."""
    n_id = 0
    for fn in nc.m.functions:
        for bb in fn.blocks:
            insts = list(bb.instructions)
            out = []
            changed = False
            for inst in insts:
                si = inst.sync_info
                if si is not None and len(si.on_wait) > cap:
                    waits = list(si.on_wait)
                    n_extra = len(waits) - cap
                    for i in range(0, n_extra, cap):
                        nop = mybir.InstNoOp(
                            name=f"waitsplit_{n_id}",
                            engine=inst.engine,
                            bass_nofuse=True,
                            sync_info=mybir.SyncInfo(
                                on_wait=list(waits[i : i + cap]), on_update=[]
                            ),
                        )
                        n_id += 1
                        nc.register_instruction(nop)
                        out.append(nop)
                    si.on_wait = waits[n_extra:]
                    changed = True
                out.append(inst)
            if changed:
                bb.instructions = out


# Patch 2: NTFF profile hook (only needed when tracing; the image's antenv
# lacks axon_hooks, so register a ctypes-based hook ourselves).
def _install_ntff_hook():
    try:
        from antenv.axon_hooks import get_axon_ntff_profile_hook  # noqa: F401

        return
    except ImportError:
        pass
    try:
        from trn_agent_boot.trn_boot import _ntff_profile_via_ctypes

        hook = _ntff_profile_via_ctypes("/opt/axon/libaxon_pjrt.so")
    except Exception:
        hook = None
    import antenv

    mod = types.ModuleType("antenv.axon_hooks")
    mod.get_axon_ntff_profile_hook = lambda: hook
    sys.modules["antenv.axon_hooks"] = mod
    antenv.axon_hooks = mod


# ---------------------------------------------------------------------------
# Device kernel builder


def _build_nc(Uc, mode, chunks_per_dma, wp_bufs=WP_BUFS):
    """Build the per-core Bass program.

    hilo mode stacks W_hi then W_lo along the contraction axis (128
    k-chunks; the stationary x2 = [x_hi | x_lo] repeats for both halves),
    so PSUM accumulates all four bf16 cross products: psum[0:32] =
    x_hi@(W_hi+W_lo), psum[32:64] = x_lo@(W_hi+W_lo); one DVE add
    finishes the job.

    Inputs (per core):
      w  : [G, 128, A*Uc]  moving operand (weight shard, transposed)
           G*A = n_chunks k-chunks of 128 rows; Uc columns each.
      x2 : [128, 64*M]     stationary operand; chunk kb uses cols (kb%64)*M.
    Output:
      f  : [32, Uc] f32    f[o*16+t, u]
    """
    A = chunks_per_dma
    if mode == "hilo":
        M = 64
        n_chunks = 128
        dt_in = mybir.dt.bfloat16
    elif mode == "f32":
        M = 32
        n_chunks = 64
        dt_in = mybir.dt.float32
    elif mode == "f32r":
        M = 32
        n_chunks = 64
        dt_in = mybir.dt.float32r
    elif mode == "fp16":
        M = 32
        n_chunks = 64
        dt_in = mybir.dt.float16
    elif mode == "mix3":
        return _build_nc_mix3(Uc, chunks_per_dma, wp_bufs)
    else:
        raise ValueError(mode)
    G = n_chunks // A
    C = Uc
    NB = 512
    n_blocks = (C + NB - 1) // NB

    nc = bass.Bass("TRN2", target_bir_lowering=False, debug=False,
                   num_devices=N_CORES)
    w = nc.dram_tensor("w", [G, 128, A * C], dt_in, kind="ExternalInput").ap()
    x2 = nc.dram_tensor("x2", [128, 64 * M], dt_in,
                        kind="ExternalInput").ap()
    f = nc.dram_tensor("f", [M, Uc], mybir.dt.float32,
                       kind="ExternalOutput").ap()

    with tile.TileContext(nc) as tc:
        with (
            tc.tile_pool(name="x2p", bufs=1) as x2p,
            tc.tile_pool(name="wp", bufs=wp_bufs) as wp,
            tc.tile_pool(name="psum", bufs=1, space="PSUM") as pp,
            tc.tile_pool(name="outp", bufs=1) as outp,
        ):
            x2_t = x2p.tile([128, 64 * M], dt_in)
            nc.sync.dma_start(x2_t[:], x2[:])

            psums = [pp.tile([M, min(NB, C - nb * NB)], mybir.dt.float32,
                             name=f"ps{nb}")
                     for nb in range(n_blocks)]
            out_t = outp.tile([M, Uc], mybir.dt.float32)

            for g in range(G):
                w_t = wp.tile([128, A * C], dt_in, tag="w")
                # alternate the two HWDGE queues (ACT / SP); w0 goes on ACT
                # so it transfers concurrently with x2 (which is on SP).
                eng = nc.scalar if g % 2 == 0 else nc.sync
                eng.dma_start(w_t[:], w[g, :, :])
                for a in range(A):
                    kb = g * A + a
                    j = kb % 64
                    lhsT = x2_t[:, j * M : (j + 1) * M]
                    for nb in range(n_blocks):
                        w_nb = min(NB, C - nb * NB)
                        rhs = w_t[:, a * C + nb * NB : a * C + nb * NB + w_nb]
                        nc.tensor.matmul(
                            psums[nb][:, :],
                            lhsT,
                            rhs,
                            start=(kb == 0),
                            stop=(kb == n_chunks - 1),
                        )

            # Stage psums to SBUF; the halves-sum happens on the host.
            for nb in range(n_blocks):
                w_nb = min(NB, C - nb * NB)
                sl = slice(nb * NB, nb * NB + w_nb)
                nc.vector.tensor_copy(out_t[:, sl], psums[nb][:, :])
            nc.sync.dma_start(f[:], out_t[:])
    _split_waits(nc)
    return nc


MIX3_SCALE = 2.0 ** 18


def _build_nc_mix3(Uc, chunks_per_dma, wp_bufs):
    """3-byte mixed precision: W = fp16(W) + 2^-18 * fp8e4m3(scaled resid).

    hi half: 64 fp16 chunks, stationary [x_hi | x_lo] fp16 (M=64).
    lo half: 64 fp8e4m3 chunks, stationary x fp8e4m3 (M=32), psum scaled
    by 2^-18 during the combine.
    """
    A = chunks_per_dma
    A8 = 2 * A
    G = 64 // A
    G8 = 64 // A8
    C = Uc
    NB = 512
    n_blocks = (C + NB - 1) // NB

    nc = bass.Bass("TRN2", target_bir_lowering=False, debug=False,
                   num_devices=N_CORES)
    wh = nc.dram_tensor("wh", [G, 128, A * C], mybir.dt.float16,
                        kind="ExternalInput").ap()
    wl = nc.dram_tensor("wl", [G8, 128, A8 * C], mybir.dt.float8e4,
                        kind="ExternalInput").ap()
    x2h = nc.dram_tensor("x2h", [128, 64 * 64], mybir.dt.float16,
                         kind="ExternalInput").ap()
    x2l = nc.dram_tensor("x2l", [128, 64 * 32], mybir.dt.float8e4,
                         kind="ExternalInput").ap()
    f = nc.dram_tensor("f", [96, Uc], mybir.dt.float32,
                       kind="ExternalOutput").ap()

    with tile.TileContext(nc) as tc:
        with (
            tc.tile_pool(name="x2p", bufs=1) as x2p,
            tc.tile_pool(name="wp", bufs=wp_bufs) as wp,
            tc.tile_pool(name="wp8", bufs=2) as wp8,
            tc.tile_pool(name="psum", bufs=1, space="PSUM") as pp,
            tc.tile_pool(name="outp", bufs=1) as outp,
        ):
            x2h_t = x2p.tile([128, 64 * 64], mybir.dt.float16, name="x2h_t")
            nc.sync.dma_start(x2h_t[:], x2h[:])
            x2l_t = x2p.tile([128, 64 * 32], mybir.dt.float8e4, name="x2l_t")
            nc.sync.dma_start(x2l_t[:], x2l[:])

            psH = [pp.tile([64, min(NB, C - nb * NB)], mybir.dt.float32,
                           name=f"psH{nb}") for nb in range(n_blocks)]
            psL = [pp.tile([32, min(NB, C - nb * NB)], mybir.dt.float32,
                           name=f"psL{nb}") for nb in range(n_blocks)]
            out_t = outp.tile([96, Uc], mybir.dt.float32)

            # Interleave hi (ACT queue) and lo (SP queue) DMA groups.
            # hi group g covers chunks [g*A, (g+1)*A); lo group covers 2A.
            for g in range(G):
                wh_t = wp.tile([128, A * C], mybir.dt.float16, tag="wh")
                heng = nc.scalar if g % 2 == 0 else nc.sync
                heng.dma_start(wh_t[:], wh[g, :, :])
                if g % 2 == 0:
                    g8 = g // 2
                    wl_t = wp8.tile([128, A8 * C], mybir.dt.float8e4,
                                    tag="wl")
                    leng = nc.sync if g % 2 == 0 else nc.scalar
                    leng.dma_start(wl_t[:], wl[g8, :, :])
                for a in range(A):
                    kb = g * A + a
                    lhsT = x2h_t[:, kb * 64 : (kb + 1) * 64]
                    for nb in range(n_blocks):
                        w_nb = min(NB, C - nb * NB)
                        rhs = wh_t[:, a * C + nb * NB : a * C + nb * NB + w_nb]
                        nc.tensor.matmul(psH[nb][:, :], lhsT, rhs,
                                         start=(kb == 0), stop=(kb == 63))
                if g % 2 == 1:
                    # wl_t holds chunks [ (g//2)*2A, (g//2+1)*2A ).
                    # DoubleRow: process chunk PAIRS (contraction 256) with
                    # 2 fp8 weights per PE cell.
                    n_pairs_per_tile = A8 // 2
                    for a8p in range(n_pairs_per_tile):
                        pair = (g // 2) * n_pairs_per_tile + a8p
                        lhsT8 = x2l_t[
                            :, (2 * pair) * 32 : (2 * pair + 2) * 32
                        ].rearrange("p (c m) -> p c m", c=2)
                        rhs_pair = wl_t[
                            :, (2 * a8p) * C : (2 * a8p + 2) * C
                        ].rearrange("p (c n) -> p c n", c=2)
                        for nb in range(n_blocks):
                            w_nb = min(NB, C - nb * NB)
                            rhs = rhs_pair[:, :, nb * NB : nb * NB + w_nb]
                            nc.tensor.matmul(
                                psL[nb][:, :], lhsT8, rhs,
                                start=(pair == 0), stop=(pair == 31),
                                perf_mode=mybir.MatmulPerfMode.DoubleRow,
                            )

            # Stage psums to SBUF (DVE for the hi halves, ACT for lo, in
            # parallel); descale + sum happen on the host.
            for nb in range(n_blocks):
                w_nb = min(NB, C - nb * NB)
                sl = slice(nb * NB, nb * NB + w_nb)
                nc.vector.tensor_copy(out_t[0:64, sl], psH[nb][:, :])
                nc.scalar.copy(out_t[64:96, sl], psL[nb][:, :])
            nc.sync.dma_start(f[:], out_t[:])
    _split_waits(nc)
    return nc


_NC_CACHE = {}


def _get_nc(Uc, mode, chunks_per_dma):
    key = (Uc, mode, chunks_per_dma)
    if key not in _NC_CACHE:
        _NC_CACHE[key] = _build_nc(Uc, mode, chunks_per_dma)
    return _NC_CACHE[key]


# ---------------------------------------------------------------------------
# Host side


def _split_hilo(arr_f32):
    hi = arr_f32.astype(BF16)
    lo = (arr_f32 - hi.astype(np.float32)).astype(BF16)
    return hi, lo


def _prepare(x, weight, node_in, top, bottom, left, right, mode, A):
    """Host prep: dedup indices, build per-core input maps. Returns
    (in_maps, meta)."""
    T, three, Mdim = x.shape
    assert three == 3
    N, K4, two = weight.shape
    K = K4  # 4*M

    # ---- dedup + shard -----------------------------------------------------
    idx_all = np.concatenate([node_in, top, bottom, left, right]) - 1
    uniq, inv = np.unique(idx_all, return_inverse=True)
    U0 = len(uniq)
    Uc = -(-U0 // N_CORES)
    Uc = ((Uc + 15) // 16) * 16  # pad (16: DoubleRow AP stride % 16 == 0)
    Upad = Uc * N_CORES
    uniq_pad = np.zeros(Upad, dtype=np.int64)
    uniq_pad[:U0] = uniq

    # ---- x_out and stationary operand -------------------------------------
    xo = np.stack([x[:, 0, :], x[:, 2, :], x[:, 2, :], x[:, 1, :]],
                  axis=-1).reshape(T, K)  # [T, 4M]
    K2 = 2 * K  # interleaved contraction k' = 2i + o
    K_CHUNKS = K2 // 128
    assert K2 == K_CHUNKS * 128

    def _stationary(parts, np_dt):
        """Build [128, 64 * 16*len(parts)*2] chunk-grouped stationary from a
        list of [T, K] matrices (each gets o=0/o=1 zero-interleaved cols)."""
        M = 2 * T * len(parts)
        X2 = np.zeros((K2, M), dtype=np_dt)
        for h, p in enumerate(parts):
            X2[0::2, 2 * h * T : (2 * h + 1) * T] = p.T
            X2[1::2, (2 * h + 1) * T : (2 * h + 2) * T] = p.T
        return np.ascontiguousarray(
            X2.reshape(K_CHUNKS, 128, M).transpose(1, 0, 2).reshape(
                128, K_CHUNKS * M)
        )

    FP8 = ml_dtypes.float8_e4m3
    if mode == "hilo":
        x_hi, x_lo = _split_hilo(xo)
        x2r = _stationary([x_hi, x_lo], BF16)
    elif mode == "mix3":
        xh16 = xo.astype(np.float16)
        xl16 = (xo - xh16.astype(np.float32)).astype(np.float16)
        x2r_h = _stationary([xh16, xl16], np.float16)
        x2r_l = _stationary([xo.astype(FP8)], FP8)
    elif mode == "fp16":
        x2r = _stationary([xo.astype(np.float16)], np.float16)
    else:
        x2r = _stationary([xo], np.float32)

    # ---- per-core weight shards -------------------------------------------
    def _grouped(V_T_src, A_):
        """[Uc, Kt] value matrix -> chunk-grouped [G, 128, A_*Uc]."""
        Kt = V_T_src.shape[1]
        G_ = Kt // 128 // A_
        return np.ascontiguousarray(
            V_T_src.T.reshape(G_, A_, 128, Uc).transpose(0, 2, 1, 3).reshape(
                G_, 128, A_ * Uc
            )
        )

    wf = weight.reshape(N, K2)  # row n: k' = 2i+o contiguous
    in_maps = []
    for c in range(N_CORES):
        rows = uniq_pad[c * Uc : (c + 1) * Uc]
        Wg = wf[rows]  # [Uc, K2] f32
        if mode == "hilo":
            hi, lo = _split_hilo(Wg)
            V = np.concatenate([hi, lo], axis=1)  # [Uc, 2*K2]
            in_maps.append({"w": _grouped(V, A), "x2": x2r})
        elif mode == "mix3":
            wh16 = Wg.astype(np.float16)
            wl8 = ((Wg - wh16.astype(np.float32)) * MIX3_SCALE).astype(FP8)
            in_maps.append({
                "wh": _grouped(wh16, A),
                "wl": _grouped(wl8, 2 * A),
                "x2h": x2r_h,
                "x2l": x2r_l,
            })
        elif mode == "fp16":
            in_maps.append({"w": _grouped(Wg.astype(np.float16), A),
                            "x2": x2r})
        else:
            in_maps.append({"w": _grouped(Wg, A), "x2": x2r})

    meta = dict(T=T, Uc=Uc, Upad=Upad, inv=inv, mode=mode, A=A)
    return in_maps, meta


def _assemble(per_core_f, meta, node_in, top):
    """Unshard: build f_uniq, gather f_in, boundary sums."""
    T, Uc, Upad, inv = meta["T"], meta["Uc"], meta["Upad"], meta["inv"]
    n_in = node_in.shape[0]
    n_b = top.shape[0]
    mode = meta["mode"]
    f_uniq = np.empty((Upad, T, 2), dtype=np.float32)
    for c in range(N_CORES):
        fc = per_core_f[c]  # [M, Uc]: rows (part, o*T+t)
        if mode == "mix3":
            fc = fc[0:32] + fc[32:64] + fc[64:96] * np.float32(1.0 /
                                                               MIX3_SCALE)
        elif mode == "hilo":
            fc = fc[0:32] + fc[32:64]
        f_uniq[c * Uc : (c + 1) * Uc] = fc.reshape(2, T, Uc).transpose(2, 1, 0)

    f_in = np.ascontiguousarray(f_uniq[inv[:n_in]])
    bi = inv[n_in:]
    s_top = f_uniq[bi[0:n_b], :, 1].sum(axis=0)
    s_bot = f_uniq[bi[n_b : 2 * n_b], :, 1].sum(axis=0)
    s_left = f_uniq[bi[2 * n_b : 3 * n_b], :, 0].sum(axis=0)
    s_right = f_uniq[bi[3 * n_b : 4 * n_b], :, 0].sum(axis=0)
    f_b = np.stack(
        [s_top, s_bot, s_left, s_right, s_top + s_bot, s_left + s_right],
        axis=0,
    )[..., None].astype(np.float32)
    return f_in, f_b


def kernel(x, weight, node_in, top, bottom, left, right):
    global LAST_EXEC_NS, LAST_RESULTS
    x = np.asarray(x, dtype=np.float32)
    weight = np.asarray(weight, dtype=np.float32)
    node_in = np.asarray(node_in).astype(np.int64)
    top = np.asarray(top).astype(np.int64)
    bottom = np.asarray(bottom).astype(np.int64)
    left = np.asarray(left).astype(np.int64)
    right = np.asarray(right).astype(np.int64)

    in_maps, meta = _prepare(x, weight, node_in, top, bottom, left, right,
                             MODE, CHUNKS_PER_DMA)
    nc = _get_nc(meta["Uc"], MODE, CHUNKS_PER_DMA)
    if TRACE:
        _install_ntff_hook()
        import tempfile

        res = run_bass_kernel_spmd(
            nc, in_maps, list(range(N_CORES)), trace=True,
            tmpdir=tempfile.mkdtemp(prefix="eq_trace_"),
        )
        LAST_EXEC_NS = res.exec_time_ns
    else:
        res = run_bass_kernel_spmd(nc, in_maps, list(range(N_CORES)))
    LAST_RESULTS = res

    per_core_f = [res.results[c]["f"] for c in range(N_CORES)]
    return _assemble(per_core_f, meta, node_in, top)
